# revision 42
# baseline (speedup 1.0000x reference)
"""Trainium2 Bass kernel for DCTLAVISBlip dc_transform (DCT -> truncate -> IDCT).

Math (symmetry-folded, from v2): DCT parity M[k, T-1-t] = (-1)^k M[k,t]
folds the input on the host (u = xf+xr, v = xf-xr), halving the MACs.
Device runs Wu = [Me; pad; Pe'] and Wv = [Mo; pad; Po'] ([575, 288])
against u/v; y rows and raw a/b state halves ship as f16; the host does
the row interleave and the a+-b combine.  ~119-120us vs the 130us v2
baseline; PE busy ~95us of it (pass-count is within ~10% of the
M-row x K-tile lower bound for this shape, and deeper DCT factorization
levels fragment on the 128-lane granularity -- measured matmul cost is
flat ~240ns/512-col pass for any K<=128, so only pass count matters).

DMA/schedule structure (what the iterations v3-v9 established):
  1. v ships as fp8 e3m4 (4 mantissa bits), u as f16.  The PE accepts
     mixed f16-weight x fp8-moving matmuls; error goes 7e-4 -> 1.3e-2
     (tolerance 2e-2).  Both-sides e3m4 measured 2.1e-2 -- just over.
     Halves the v input bytes; v runs FIRST so the cheap kicks open the
     pipeline.
  2. ONE output DRAM tensor os[2, 2, 576, 4C] (wave-row x 4-batch
     layout, stage cols (ni, bi, 512)): ONE dma_start per (q, t,
     m-tile) = 20 calls of ~1MB with 8KB-contiguous DRAM lines.
     DIRECT2D issue costs 0.6-3us per call on a sequencer, so fewer,
     fatter calls beat many small ones; descriptors of one call fan
     out across all 16 SDMA engines.
  3. Output issue alternates the sync and gpsimd rings; every third
     call goes via the scalar (Act) ring DELAYED one wave, so its
     issue never blocks the scalar drain copies (that coupling cost
     v3 ~16us of PSUM stalls).  The last 4 waves ship each n-half as
     soon as its drains finish, across both free rings.
  4. Inputs stream on the scalar ring (weights on sync, v-first);
     the first wave's two kicks ship whole tiles (4KB-contiguous
     descriptor lines -- 1KB column-kicks measured descriptor-bound).
  5. PE warmup (memset + 18 matmuls) covers the HAM clock-gate window
     (~3.4us) during the input DMA head; first m-tile ramps in
     2-batch PSUM groups.  K=288 = 2x128 + 32-row remainder, the
     remainders of 4 batches packed on one 128-partition tile and
     co-executed on PE row-quarters via tile_position (the 4-way
     group costs ~1 pass instead of 4).
"""

import numpy as np

B, T, C = 64, 576, 1024
H = T // 2                   # 288, folded K
NCORES = 8
BPC = B // NCORES            # batches per core
Q = 0.8

_CACHED = {}


def _dct_mat(N):
    n = np.arange(N)
    Mm = np.cos(np.pi * (2 * n[None, :] + 1) * n[:, None] / (2 * N))
    s = np.full(N, np.sqrt(2.0 / N))
    s[0] = np.sqrt(1.0 / N)
    return s[:, None] * Mm          # float64


def _build_weights(L):
    """Wu [H+ns1, 288] = [Me; pad; Pe'], Wv [H+ns2, 288] = [Mo; pad; Po'].
    The y block is zero-padded up to H=288 rows so the state block starts at
    a 32-aligned PSUM partition in every m-tile."""
    M64 = _dct_mat(T)
    Mi = _dct_mat(L)
    ke = np.arange(0, L, 2)
    ko = np.arange(1, L, 2)
    Pe = np.einsum('kj,kt->jt', Mi[ke, :], M64[ke, :])
    Po = np.einsum('kj,kt->jt', Mi[ko, :], M64[ko, :])
    ns1 = (L + 1) // 2
    ns2 = L // 2
    pe_u = np.zeros((H - len(ke), H))
    pe_v = np.zeros((H - len(ko), H))
    Wu = np.concatenate([M64[ke][:, :H], pe_u, Pe[:ns1, :H]], axis=0)
    Wv = np.concatenate([M64[ko][:, :H], pe_v, Po[:ns2, :H]], axis=0)
    return Wu, Wv


def _build_nc(L):
    """Bass program for truncation length L (574 for the seed-0 input).

    Inputs host-packed as in v2:
      xu/xv  [2, 2, 128, 4C] f16: (q, ki, p, (b c))
      xur/xvr [2, 128, C]: K-remainder rows of 4 batches packed on partitions
      wub/wvb [128, 2M]: cols (ki m); wur/wvr [128, M]: rem rows 4x-replic.
    Outputs (v3): yy/ss [2, L, 4, C] f16 -- quad-major so one dma_start per
    (q, t, m-tile, dest) ships 4 batches with 8KB-contiguous DRAM lines.
    """
    import concourse.bacc as bacc
    import concourse.mybir as mybir
    import concourse.tile as tile

    f16 = mybir.dt.float16
    f32 = mybir.dt.float32

    ns1 = (L + 1) // 2
    ns2 = L // 2
    MU = H + ns1
    MV = H + ns2
    MW = {"u": MU, "v": MV}
    YB = {"u": ns1, "v": ns2}         # y rows per transform
    NT = [(0, 512), (512, 512)]
    MM = max(MU, MV)
    MT = [(m0, min(128, MM - m0)) for m0 in range(0, MM, 128)]

    f8 = mybir.dt.float8e3

    nc = bacc.Bacc("TRN2", target_bir_lowering=False, debug=False,
                   num_devices=NCORES)
    xu = nc.dram_tensor("xu", [2, 2, 128, 4 * C], f16, kind="ExternalInput")
    xv = nc.dram_tensor("xv", [2, 2, 128, 4 * C], f8, kind="ExternalInput")
    xur = nc.dram_tensor("xur", [2, 128, C], f16, kind="ExternalInput")
    xvr = nc.dram_tensor("xvr", [2, 128, C], f8, kind="ExternalInput")
    wub = nc.dram_tensor("wub", [128, 2 * MU], f16, kind="ExternalInput")
    wvb = nc.dram_tensor("wvb", [128, 2 * MV], f16, kind="ExternalInput")
    wur = nc.dram_tensor("wur", [128, MU], f16, kind="ExternalInput")
    wvr = nc.dram_tensor("wvr", [128, MV], f16, kind="ExternalInput")
    # one output tensor: plane t=0 holds [y-u rows; pad; a rows], t=1 holds
    # [y-v rows; pad; b rows] -- one dma_start per (q, t, m-tile)
    os_ = nc.dram_tensor("os", [2, 2, 576, 4 * C], f16,
                         kind="ExternalOutput")
    XD = {"u": (xu, xur, wub, wur), "v": (xv, xvr, wvb, wvr)}

    with tile.TileContext(nc) as tc:
        with (
            tc.tile_pool(name="wpool", bufs=1) as wpool,
            tc.tile_pool(name="xpool", bufs=1) as xpool,
            tc.tile_pool(name="opool", bufs=8) as opool,
            tc.tile_pool(name="ps", bufs=8, space="PSUM") as ps,
        ):
            # --- warmup immediately: memset on vector (idle at start) ---
            wz = wpool.tile([128, 128], f16, tag="wz", name="wz")
            nc.vector.memset(wz[:], 0.0)
            pwarm = ps.tile([128, 512], f32, tag="pt", name="pt")
            for _ in range(14):
                nc.tensor.matmul(pwarm[:, 0:128], wz[:], wz[:],
                                 start=True, stop=True)

            # --- input kicks, first-use order, spread across engines ---
            xt, rt, wt, wr = {}, {}, {}, {}

            def load_w(t, eng):
                _, _, wd, wrd = XD[t]
                w_ = wpool.tile([128, 2 * MW[t]], f16, tag=f"w{t}",
                                name=f"w{t}")
                eng.dma_start(w_[:], wd[:, :])
                wt[t] = w_
                w_ = wpool.tile([128, MW[t]], f16, tag=f"w{t}r",
                                name=f"w{t}r")
                eng.dma_start(w_[:], wrd[:, :])
                wr[t] = w_

            load_w("v", nc.sync)     # v runs first
            # inputs mostly on the Act (scalar) HWDGE ring; the FIRST
            # wave's (q0, v) kicks go fine-grained across all 3 rings so
            # compute starts as early as possible
            # first wave's (q0, v) kicks: ONE whole-tile kick per ki with
            # 4KB-contiguous lines (fat descriptors beat many small ones;
            # 1KB column-kicks measured descriptor-rate-bound)
            xdt = {"u": f16, "v": f8}
            xd, rd, _, _ = XD["v"]
            for ki, eng in ((0, nc.scalar), (1, nc.sync)):
                x_ = xpool.tile([128, 4 * C], f8, tag=f"xv0{ki}",
                                name=f"xv0{ki}")
                xt[("v", 0, ki)] = x_
                eng.dma_start(x_[:], xd[0, ki, :, :])
            r_ = xpool.tile([128, C], f8, tag="xvr0", name="xvr0")
            nc.gpsimd.dma_start(r_[:], rd[0, :, :])
            rt[("v", 0)] = r_
            load_w("u", nc.sync)
            for t in ("v", "u"):
                for q in range(2):
                    if q == 0 and t == "v":
                        continue
                    xd, rd, _, _ = XD[t]
                    for ki in range(2):
                        x_ = xpool.tile([128, 4 * C], xdt[t],
                                        tag=f"x{t}{q}{ki}",
                                        name=f"x{t}{q}{ki}")
                        nc.scalar.dma_start(x_[:, 0:2 * C],
                                            xd[q, ki, :, 0:2 * C])
                        xt[(t, q, ki)] = x_
                    r_ = xpool.tile([128, C], xdt[t], tag=f"x{t}r{q}",
                                    name=f"x{t}r{q}")
                    nc.gpsimd.dma_start(r_[:], rd[q, :, :])
                    rt[(t, q)] = r_
                    for ki in range(2):
                        nc.scalar.dma_start(xt[(t, q, ki)][:, 2 * C:4 * C],
                                            xd[q, ki, :, 2 * C:4 * C])

            def vcopy(dst, src):
                nc.vector.tensor_copy(dst, src)

            def scopy(dst, src):
                nc.scalar.copy(dst, src)

            oengs = [nc.sync, nc.gpsimd]
            ok_i = 0     # output call counter (engine rotation)
            pending = None   # delayed output call issued via scalar ring

            # --- compute waves: (t, q, m), 2 n-halves x 4 batches ---
            # BOTH v quads first: the first 10 waves need only 2.35MB of
            # fp8 input, while all of u (f16) streams during v's compute
            NW = 20          # total waves
            for t in ("v", "u"):
                for q in range(2):
                    ti = 0 if t == "u" else 1
                    mw = MW[t]
                    for mi, (m0, mm) in enumerate(MT):
                        mmt = min(mm, mw - m0)
                        if mmt <= 0:
                            continue
                        stage = opool.tile([128, 4 * C], f16,
                                           tag="o", name="o")
                        for ni, (n0, nn) in enumerate(NT):
                            # ramp: first m-tile of the run goes in 2-bank
                            # halves so compute starts on half the inputs
                            groups = ([(0, 1), (2, 3)]
                                      if (q == 0 and t == "v" and mi == 0)
                                      else [(0, 1, 2, 3)])
                            pts = {}
                            for grp in groups:
                                for bi in grp:
                                    pts[bi] = ps.tile([128, 512], f32,
                                                      tag="pt", name="pt")
                                for ki in range(2):
                                    wsl = wt[t][:, ki * mw + m0:
                                                ki * mw + m0 + mmt]
                                    for bi in grp:
                                        nc.tensor.matmul(
                                            pts[bi][0:mmt, :],
                                            wsl,
                                            xt[(t, q, ki)][:, bi * C + n0:
                                                           bi * C + n0 + nn],
                                            start=(ki == 0), stop=False)
                                for bi in grp:
                                    nc.tensor.matmul(
                                        pts[bi][0:mmt, :],
                                        wr[t][32 * bi:32 * bi + 32,
                                              m0:m0 + mmt],
                                        rt[(t, q)][32 * bi:32 * bi + 32,
                                                   n0:n0 + nn],
                                        start=False, stop=True,
                                        tile_position=(32 * bi, 0))
                            # stage columns laid out (ni, bi, 512): each
                            # n-half is contiguous, so tail waves can ship
                            # a half as soon as its drains complete
                            for bi in range(4):
                                cp = vcopy if bi % 2 == 0 else scopy
                                c0 = ni * 2 * C + bi * 512
                                cp(stage[0:mmt, c0:c0 + nn],
                                   pts[bi][0:mmt, :])
                            if ok_i >= NW - 4:
                                d = os_[q, ti, m0:m0 + mmt,
                                        ni * 2 * C:(ni + 1) * 2 * C]
                                oengs[ni % 2].dma_start(
                                    d, stage[0:mmt,
                                             ni * 2 * C:(ni + 1) * 2 * C])
                        # ONE dma_start per wave ships y+state rows of all
                        # 4 batches.  Rotation sync/gpsimd immediate; every
                        # third call goes via the scalar (Act) ring delayed
                        # ONE wave so its issue never blocks scalar drains.
                        if pending is not None:
                            nc.scalar.dma_start(*pending)
                            pending = None
                        d = os_[q, ti, m0:m0 + mmt, :]
                        if ok_i >= NW - 4:
                            pass     # shipped per n-half above
                        elif ok_i % 3 == 2:
                            pending = (d, stage[0:mmt, :])
                        else:
                            oengs[ok_i % 3].dma_start(d, stage[0:mmt, :])
                        ok_i += 1
            if pending is not None:
                nc.scalar.dma_start(*pending)
                pending = None

    nc.finalize()
    return nc


def _get_nc(L):
    key = ("nc3", L)
    if key not in _CACHED:
        _CACHED[key] = _build_nc(L)
    return _CACHED[key]


def _ensure_trace_hook_safe():
    """If BASS_TRACE is set in the environment, run_bass_kernel_spmd imports
    antenv.axon_hooks, which may not exist. Install a working ctypes-based
    shim when possible, else disable tracing so the run cannot crash."""
    import os
    import sys
    import types

    if not os.environ.get("BASS_TRACE"):
        return
    try:
        import antenv.axon_hooks  # noqa: F401
        return
    except ImportError:
        pass
    try:
        from trn_agent_boot.trn_boot import _ntff_profile_via_ctypes
        hooks = types.ModuleType("antenv.axon_hooks")
        hook = _ntff_profile_via_ctypes("/opt/axon/libaxon_pjrt.so")
        hooks.get_axon_ntff_profile_hook = lambda: hook
        hooks.set_axon_ntff_profile_hook = lambda h: None
        sys.modules["antenv.axon_hooks"] = hooks
    except Exception:
        os.environ["BASS_NEVER_TRACE"] = "1"


def kernel(x: np.ndarray):
    from concourse.bass_utils import run_bass_kernel_spmd

    _ensure_trace_hook_safe()
    x = np.ascontiguousarray(np.asarray(x, dtype=np.float32))
    assert x.shape == (B, T, C)

    # ---- host: data-dependent truncation length L (tiny, exact math) ----
    M64 = _dct_mat(T)
    xbar = x.astype(np.float64).mean(axis=(0, 2))
    vq = np.abs(M64 @ xbar)
    thr = np.abs(np.quantile(vq, Q))
    idxs = np.where(vq > thr)[0]
    last_index = int(idxs[-1]) if idxs.size > 0 else -1
    L = last_index if last_index >= 0 else T - 1

    ns1 = (L + 1) // 2
    Wu, Wv = _build_weights(L)              # [H+ns1, 288], [H+ns2, 288]
    wu16 = np.ascontiguousarray(Wu.T).astype(np.float16)   # [288, H+ns1]
    wv16 = np.ascontiguousarray(Wv.T).astype(np.float16)

    # ---- host: fold input (u ships f16, v ships fp8 e3m4) ----
    import ml_dtypes
    xf = x[:, :H, :]
    xr = x[:, T - 1:H - 1:-1, :]
    u16 = (xf + xr).astype(np.float16)
    v16 = (xf - xr).astype(ml_dtypes.float8_e3m4)

    nc = _get_nc(L)

    def pack_x(z16):
        # [BPC,288,C] -> [2,2,128,4C] (q, ki, p, (b c)) + rem [2,128,C]
        full = z16[:, :256].reshape(2, 4, 2, 128, C)
        full = np.ascontiguousarray(full.transpose(0, 2, 3, 1, 4)
                                    ).reshape(2, 2, 128, 4 * C)
        remn = np.ascontiguousarray(z16[:, 256:288]).reshape(2, 128, C)
        return full, remn

    def pack_w(w16):
        # [288, M] -> [128, 2M] cols (ki m) + rem rows replicated [128, M]
        full = np.ascontiguousarray(w16[:256].reshape(2, 128, w16.shape[1])
                                    .transpose(1, 0, 2)
                                    ).reshape(128, 2 * w16.shape[1])
        remn = np.ascontiguousarray(np.tile(w16[256:288], (4, 1)))
        return full, remn

    wub_h, wur_h = pack_w(wu16)
    wvb_h, wvr_h = pack_w(wv16)
    in_maps = []
    for i in range(NCORES):
        xu_h, xur_h = pack_x(u16[i * BPC:(i + 1) * BPC])
        xv_h, xvr_h = pack_x(v16[i * BPC:(i + 1) * BPC])
        in_maps.append({"xu": xu_h, "xv": xv_h, "xur": xur_h, "xvr": xvr_h,
                        "wub": wub_h, "wvb": wvb_h,
                        "wur": wur_h, "wvr": wvr_h})
    res = run_bass_kernel_spmd(nc, in_maps, list(range(NCORES)))
    _CACHED["last_exec_time_ns"] = res.exec_time_ns

    # device layout os [2(q), 2(t), 576, 4, C]:
    #   t=0 rows [0:ns1]=y-even, [H:H+ns1]=a;  t=1 [0:ns2]=y-odd, [H:H+ns2]=b
    ns2 = L // 2

    def unq(o, tp, r0, rn):
        # stage cols (ni, bi, 512): [2, rn, 2, 4, 512] -> [BPC, rn, C]
        return o[:, tp, r0:r0 + rn, :].reshape(2, rn, 2, 4, 512) \
            .transpose(0, 3, 1, 2, 4).reshape(BPC, rn, C)

    osr = [np.asarray(res.results[i]["os"]).reshape(2, 2, 576, 4 * C)
           for i in range(NCORES)]
    ye = np.concatenate([unq(o, 0, 0, ns1) for o in osr], axis=0)
    yo = np.concatenate([unq(o, 1, 0, ns2) for o in osr], axis=0)
    aa = np.concatenate([unq(o, 0, H, ns1) for o in osr], axis=0)
    bb = np.concatenate([unq(o, 1, H, ns2) for o in osr], axis=0)

    x_dct_trunc = np.empty((B, L, C), dtype=np.float32)
    x_dct_trunc[:, 0::2, :] = ye.astype(np.float32)
    x_dct_trunc[:, 1::2, :] = yo.astype(np.float32)
    a32 = aa.astype(np.float32)
    b32 = bb.astype(np.float32)
    state = np.empty((B, L, C), dtype=np.float16)
    state[:, :ns2, :] = (a32[:, :ns2] + b32).astype(np.float16)
    if ns1 > ns2:
        state[:, ns2:ns1, :] = aa[:, ns2:ns1, :]   # lone middle row, L odd
    state[:, ns1:, :] = (a32[:, :ns2] - b32).astype(np.float16)[:, ::-1, :]
    return state, x_dct_trunc


# revision 44
# speedup vs baseline: 1.0522x; 1.0522x over previous
"""Trainium2 Bass kernel for DCTLAVISBlip dc_transform (DCT -> truncate -> IDCT).

Math (symmetry-folded, from v2): DCT parity M[k, T-1-t] = (-1)^k M[k,t]
folds the input on the host (u = xf+xr, v = xf-xr), halving the MACs.
Device runs Wu = [Me; pad; Pe'] and Wv = [Mo; pad; Po'] ([575, 288])
against u/v; y rows and raw a/b state halves ship as f16; the host does
the row interleave and the a+-b combine.  ~119-120us vs the 130us v2
baseline; PE busy ~95us of it (pass-count is within ~10% of the
M-row x K-tile lower bound for this shape, and deeper DCT factorization
levels fragment on the 128-lane granularity -- measured matmul cost is
flat ~240ns/512-col pass for any K<=128, so only pass count matters).

DMA/schedule structure (what the iterations v3-v9 established):
  1. v ships as fp8 e3m4 (4 mantissa bits), u as f16.  The PE accepts
     mixed f16-weight x fp8-moving matmuls; error goes 7e-4 -> 1.3e-2
     (tolerance 2e-2).  Both-sides e3m4 measured 2.1e-2 -- just over.
     Halves the v input bytes; v runs FIRST so the cheap kicks open the
     pipeline.
  2. ONE output DRAM tensor os[2, 2, 576, 4C] (wave-row x 4-batch
     layout, stage cols (ni, bi, 512)): ONE dma_start per (q, t,
     m-tile) = 20 calls of ~1MB with 8KB-contiguous DRAM lines.
     DIRECT2D issue costs 0.6-3us per call on a sequencer, so fewer,
     fatter calls beat many small ones; descriptors of one call fan
     out across all 16 SDMA engines.
  3. Output issue alternates the sync and gpsimd rings; every third
     call goes via the scalar (Act) ring DELAYED one wave, so its
     issue never blocks the scalar drain copies (that coupling cost
     v3 ~16us of PSUM stalls).  The last 4 waves ship each n-half as
     soon as its drains finish, across both free rings.
  4. Inputs stream on the scalar ring (weights on sync, v-first);
     the first wave's two kicks ship whole tiles (4KB-contiguous
     descriptor lines -- 1KB column-kicks measured descriptor-bound).
  5. PE warmup (memset + 18 matmuls) covers the HAM clock-gate window
     (~3.4us) during the input DMA head; first m-tile ramps in
     2-batch PSUM groups.  K=288 = 2x128 + 32-row remainder, the
     remainders of 4 batches packed on one 128-partition tile and
     co-executed on PE row-quarters via tile_position (the 4-way
     group costs ~1 pass instead of 4).
"""

import numpy as np

B, T, C = 64, 576, 1024
H = T // 2                   # 288, folded K
NCORES = 8
BPC = B // NCORES            # batches per core
Q = 0.8

_CACHED = {}


def _dct_mat(N):
    n = np.arange(N)
    Mm = np.cos(np.pi * (2 * n[None, :] + 1) * n[:, None] / (2 * N))
    s = np.full(N, np.sqrt(2.0 / N))
    s[0] = np.sqrt(1.0 / N)
    return s[:, None] * Mm          # float64


def _build_weights(L):
    """Wu [H+ns1, 288] = [Me; pad; Pe'], Wv [H+ns2, 288] = [Mo; pad; Po'].
    The y block is zero-padded up to H=288 rows so the state block starts at
    a 32-aligned PSUM partition in every m-tile."""
    M64 = _dct_mat(T)
    Mi = _dct_mat(L)
    ke = np.arange(0, L, 2)
    ko = np.arange(1, L, 2)
    Pe = np.einsum('kj,kt->jt', Mi[ke, :], M64[ke, :])
    Po = np.einsum('kj,kt->jt', Mi[ko, :], M64[ko, :])
    ns1 = (L + 1) // 2
    ns2 = L // 2
    pe_u = np.zeros((H - len(ke), H))
    pe_v = np.zeros((H - len(ko), H))
    Wu = np.concatenate([M64[ke][:, :H], pe_u, Pe[:ns1, :H]], axis=0)
    Wv = np.concatenate([M64[ko][:, :H], pe_v, Po[:ns2, :H]], axis=0)
    return Wu, Wv


def _build_nc(L):
    """Bass program for truncation length L (574 for the seed-0 input).

    Inputs host-packed as in v2:
      xu/xv  [2, 2, 128, 4C] f16: (q, ki, p, (b c))
      xur/xvr [2, 128, C]: K-remainder rows of 4 batches packed on partitions
      wub/wvb [128, 2M]: cols (ki m); wur/wvr [128, M]: rem rows 4x-replic.
    Outputs (v3): yy/ss [2, L, 4, C] f16 -- quad-major so one dma_start per
    (q, t, m-tile, dest) ships 4 batches with 8KB-contiguous DRAM lines.
    """
    import concourse.bacc as bacc
    import concourse.mybir as mybir
    import concourse.tile as tile

    f16 = mybir.dt.float16
    f32 = mybir.dt.float32

    ns1 = (L + 1) // 2
    ns2 = L // 2
    MU = H + ns1
    MV = H + ns2
    MW = {"u": MU, "v": MV}
    YB = {"u": ns1, "v": ns2}         # y rows per transform
    NT = [(0, 512), (512, 512)]
    MM = max(MU, MV)
    MT = [(m0, min(128, MM - m0)) for m0 in range(0, MM, 128)]

    f8 = mybir.dt.float8e3

    nc = bacc.Bacc("TRN2", target_bir_lowering=False, debug=False,
                   num_devices=NCORES)
    xu = nc.dram_tensor("xu", [2, 2, 128, 4 * C], f16, kind="ExternalInput")
    xv = nc.dram_tensor("xv", [2, 2, 128, 4 * C], f8, kind="ExternalInput")
    xur = nc.dram_tensor("xur", [2, 128, C], f16, kind="ExternalInput")
    xvr = nc.dram_tensor("xvr", [2, 128, C], f8, kind="ExternalInput")
    wub = nc.dram_tensor("wub", [128, 2 * MU], f16, kind="ExternalInput")
    wvb = nc.dram_tensor("wvb", [128, 2 * MV], f16, kind="ExternalInput")
    wur = nc.dram_tensor("wur", [128, MU], f16, kind="ExternalInput")
    wvr = nc.dram_tensor("wvr", [128, MV], f16, kind="ExternalInput")
    # one output tensor: plane t=0 holds [y-u rows; pad; a rows], t=1 holds
    # [y-v rows; pad; b rows] -- one dma_start per (q, t, m-tile)
    os_ = nc.dram_tensor("os", [2, 2, 576, 4 * C], f16,
                         kind="ExternalOutput")
    XD = {"u": (xu, xur, wub, wur), "v": (xv, xvr, wvb, wvr)}

    with tile.TileContext(nc) as tc:
        with (
            tc.tile_pool(name="wpool", bufs=1) as wpool,
            tc.tile_pool(name="xpool", bufs=1) as xpool,
            tc.tile_pool(name="opool", bufs=8) as opool,
            tc.tile_pool(name="ps", bufs=8, space="PSUM") as ps,
        ):
            # --- warmup immediately: memset on vector (idle at start) ---
            wz = wpool.tile([128, 128], f16, tag="wz", name="wz")
            nc.vector.memset(wz[:], 0.0)
            pwarm = ps.tile([128, 512], f32, tag="pt", name="pt")
            for _ in range(14):
                nc.tensor.matmul(pwarm[:, 0:128], wz[:], wz[:],
                                 start=True, stop=True)

            # --- input kicks, first-use order, spread across engines ---
            xt, rt, wt, wr = {}, {}, {}, {}

            def load_w(t, eng):
                _, _, wd, wrd = XD[t]
                w_ = wpool.tile([128, 2 * MW[t]], f16, tag=f"w{t}",
                                name=f"w{t}")
                eng.dma_start(w_[:], wd[:, :])
                wt[t] = w_
                w_ = wpool.tile([128, MW[t]], f16, tag=f"w{t}r",
                                name=f"w{t}r")
                eng.dma_start(w_[:], wrd[:, :])
                wr[t] = w_

            # v weights: the first m-tile's ki0 slice (32KB) ships alone so
            # the first matmul's weight dependency clears immediately; the
            # rest and the rem weights follow after the first kicks
            _, _, wvd, wvrd = XD["v"]
            wv_ = wpool.tile([128, 2 * MV], f16, tag="wv", name="wv")
            nc.sync.dma_start(wv_[:, 0:128], wvd[:, 0:128])
            wt["v"] = wv_
            # first wave's (q0, v) kicks: 2KB-contiguous-line halves in
            # first-use order (the ramp needs [0:2C] of both ki)
            xdt = {"u": f16, "v": f8}
            xd, rd, _, _ = XD["v"]
            for ki, eng in ((0, nc.scalar), (1, nc.sync)):
                x_ = xpool.tile([128, 4 * C], f8, tag=f"xv0{ki}",
                                name=f"xv0{ki}")
                xt[("v", 0, ki)] = x_
                eng.dma_start(x_[:, 0:2 * C], xd[0, ki, :, 0:2 * C])
            nc.sync.dma_start(wv_[:, 128:2 * MV], wvd[:, 128:2 * MV])
            r_ = xpool.tile([128, C], f8, tag="xvr0", name="xvr0")
            nc.gpsimd.dma_start(r_[:], rd[0, :, :])
            rt[("v", 0)] = r_
            wvr_ = wpool.tile([128, MV], f16, tag="wvr", name="wvr")
            nc.gpsimd.dma_start(wvr_[:], wvrd[:, :])
            wr["v"] = wvr_
            for ki in range(2):
                nc.scalar.dma_start(xt[("v", 0, ki)][:, 2 * C:4 * C],
                                    xd[0, ki, :, 2 * C:4 * C])
            load_w("u", nc.sync)
            for t in ("v", "u"):
                for q in range(2):
                    if q == 0 and t == "v":
                        continue
                    xd, rd, _, _ = XD[t]
                    for ki in range(2):
                        x_ = xpool.tile([128, 4 * C], xdt[t],
                                        tag=f"x{t}{q}{ki}",
                                        name=f"x{t}{q}{ki}")
                        nc.scalar.dma_start(x_[:, 0:2 * C],
                                            xd[q, ki, :, 0:2 * C])
                        xt[(t, q, ki)] = x_
                    r_ = xpool.tile([128, C], xdt[t], tag=f"x{t}r{q}",
                                    name=f"x{t}r{q}")
                    nc.gpsimd.dma_start(r_[:], rd[q, :, :])
                    rt[(t, q)] = r_
                    for ki in range(2):
                        nc.scalar.dma_start(xt[(t, q, ki)][:, 2 * C:4 * C],
                                            xd[q, ki, :, 2 * C:4 * C])

            def vcopy(dst, src):
                nc.vector.tensor_copy(dst, src)

            def scopy(dst, src):
                nc.scalar.copy(dst, src)

            oengs = [nc.sync, nc.gpsimd]
            ok_i = 0     # output call counter (engine rotation)
            pending = None   # delayed output call issued via scalar ring

            # --- compute waves: (t, q, m), 2 n-halves x 4 batches ---
            # BOTH v quads first: the first 10 waves need only 2.35MB of
            # fp8 input, while all of u (f16) streams during v's compute
            NW = 20          # total waves
            for t in ("v", "u"):
                for q in range(2):
                    ti = 0 if t == "u" else 1
                    mw = MW[t]
                    for mi, (m0, mm) in enumerate(MT):
                        mmt = min(mm, mw - m0)
                        if mmt <= 0:
                            continue
                        stage = opool.tile([128, 4 * C], f16,
                                           tag="o", name="o")
                        for ni, (n0, nn) in enumerate(NT):
                            # ramp: first m-tile of the run goes in 2-bank
                            # halves so compute starts on half the inputs
                            groups = ([(0, 1), (2, 3)]
                                      if (q == 0 and t == "v" and mi == 0)
                                      else [(0, 1, 2, 3)])
                            pts = {}
                            for grp in groups:
                                for bi in grp:
                                    pts[bi] = ps.tile([128, 512], f32,
                                                      tag="pt", name="pt")
                                for ki in range(2):
                                    wsl = wt[t][:, ki * mw + m0:
                                                ki * mw + m0 + mmt]
                                    for bi in grp:
                                        nc.tensor.matmul(
                                            pts[bi][0:mmt, :],
                                            wsl,
                                            xt[(t, q, ki)][:, bi * C + n0:
                                                           bi * C + n0 + nn],
                                            start=(ki == 0), stop=False)
                                for bi in grp:
                                    nc.tensor.matmul(
                                        pts[bi][0:mmt, :],
                                        wr[t][32 * bi:32 * bi + 32,
                                              m0:m0 + mmt],
                                        rt[(t, q)][32 * bi:32 * bi + 32,
                                                   n0:n0 + nn],
                                        start=False, stop=True,
                                        tile_position=(32 * bi, 0))
                            # stage columns laid out (ni, bi, 512): each
                            # n-half is contiguous, so tail waves can ship
                            # a half as soon as its drains complete
                            for bi in range(4):
                                cp = vcopy if bi % 2 == 0 else scopy
                                c0 = ni * 2 * C + bi * 512
                                cp(stage[0:mmt, c0:c0 + nn],
                                   pts[bi][0:mmt, :])
                                if ok_i >= NW - 4 and bi % 2 == 1:
                                    # tail waves: ship each 1KB batch-pair
                                    # chunk as soon as its 2 drains land
                                    cc = ni * 2 * C + (bi - 1) * 512
                                    d = os_[q, ti, m0:m0 + mmt,
                                            cc:cc + 2 * 512]
                                    oengs[(2 * ni + bi // 2) % 2].dma_start(
                                        d, stage[0:mmt, cc:cc + 2 * 512])
                        # ONE dma_start per wave ships y+state rows of all
                        # 4 batches.  Rotation sync/gpsimd immediate; every
                        # third call goes via the scalar (Act) ring delayed
                        # ONE wave so its issue never blocks scalar drains.
                        if pending is not None:
                            nc.scalar.dma_start(*pending)
                            pending = None
                        d = os_[q, ti, m0:m0 + mmt, :]
                        if ok_i >= NW - 4:
                            pass     # shipped per n-half above
                        elif ok_i % 3 == 2:
                            pending = (d, stage[0:mmt, :])
                        else:
                            oengs[ok_i % 3].dma_start(d, stage[0:mmt, :])
                        ok_i += 1
            if pending is not None:
                nc.scalar.dma_start(*pending)
                pending = None

    nc.finalize()
    return nc


def _get_nc(L):
    key = ("nc3", L)
    if key not in _CACHED:
        _CACHED[key] = _build_nc(L)
    return _CACHED[key]


def _ensure_trace_hook_safe():
    """If BASS_TRACE is set in the environment, run_bass_kernel_spmd imports
    antenv.axon_hooks, which may not exist. Install a working ctypes-based
    shim when possible, else disable tracing so the run cannot crash."""
    import os
    import sys
    import types

    if not os.environ.get("BASS_TRACE"):
        return
    try:
        import antenv.axon_hooks  # noqa: F401
        return
    except ImportError:
        pass
    try:
        from trn_agent_boot.trn_boot import _ntff_profile_via_ctypes
        hooks = types.ModuleType("antenv.axon_hooks")
        hook = _ntff_profile_via_ctypes("/opt/axon/libaxon_pjrt.so")
        hooks.get_axon_ntff_profile_hook = lambda: hook
        hooks.set_axon_ntff_profile_hook = lambda h: None
        sys.modules["antenv.axon_hooks"] = hooks
    except Exception:
        os.environ["BASS_NEVER_TRACE"] = "1"


def kernel(x: np.ndarray):
    from concourse.bass_utils import run_bass_kernel_spmd

    _ensure_trace_hook_safe()
    x = np.ascontiguousarray(np.asarray(x, dtype=np.float32))
    assert x.shape == (B, T, C)

    # ---- host: data-dependent truncation length L (tiny, exact math) ----
    M64 = _dct_mat(T)
    xbar = x.astype(np.float64).mean(axis=(0, 2))
    vq = np.abs(M64 @ xbar)
    thr = np.abs(np.quantile(vq, Q))
    idxs = np.where(vq > thr)[0]
    last_index = int(idxs[-1]) if idxs.size > 0 else -1
    L = last_index if last_index >= 0 else T - 1

    ns1 = (L + 1) // 2
    Wu, Wv = _build_weights(L)              # [H+ns1, 288], [H+ns2, 288]
    wu16 = np.ascontiguousarray(Wu.T).astype(np.float16)   # [288, H+ns1]
    wv16 = np.ascontiguousarray(Wv.T).astype(np.float16)

    # ---- host: fold input (u ships f16, v ships fp8 e3m4) ----
    import ml_dtypes
    xf = x[:, :H, :]
    xr = x[:, T - 1:H - 1:-1, :]
    u16 = (xf + xr).astype(np.float16)
    v16 = (xf - xr).astype(ml_dtypes.float8_e3m4)

    nc = _get_nc(L)

    def pack_x(z16):
        # [BPC,288,C] -> [2,2,128,4C] (q, ki, p, (b c)) + rem [2,128,C]
        full = z16[:, :256].reshape(2, 4, 2, 128, C)
        full = np.ascontiguousarray(full.transpose(0, 2, 3, 1, 4)
                                    ).reshape(2, 2, 128, 4 * C)
        remn = np.ascontiguousarray(z16[:, 256:288]).reshape(2, 128, C)
        return full, remn

    def pack_w(w16):
        # [288, M] -> [128, 2M] cols (ki m) + rem rows replicated [128, M]
        full = np.ascontiguousarray(w16[:256].reshape(2, 128, w16.shape[1])
                                    .transpose(1, 0, 2)
                                    ).reshape(128, 2 * w16.shape[1])
        remn = np.ascontiguousarray(np.tile(w16[256:288], (4, 1)))
        return full, remn

    wub_h, wur_h = pack_w(wu16)
    wvb_h, wvr_h = pack_w(wv16)
    in_maps = []
    for i in range(NCORES):
        xu_h, xur_h = pack_x(u16[i * BPC:(i + 1) * BPC])
        xv_h, xvr_h = pack_x(v16[i * BPC:(i + 1) * BPC])
        in_maps.append({"xu": xu_h, "xv": xv_h, "xur": xur_h, "xvr": xvr_h,
                        "wub": wub_h, "wvb": wvb_h,
                        "wur": wur_h, "wvr": wvr_h})
    res = run_bass_kernel_spmd(nc, in_maps, list(range(NCORES)))
    _CACHED["last_exec_time_ns"] = res.exec_time_ns

    # device layout os [2(q), 2(t), 576, 4, C]:
    #   t=0 rows [0:ns1]=y-even, [H:H+ns1]=a;  t=1 [0:ns2]=y-odd, [H:H+ns2]=b
    ns2 = L // 2

    def unq(o, tp, r0, rn):
        # stage cols (ni, bi, 512): [2, rn, 2, 4, 512] -> [BPC, rn, C]
        return o[:, tp, r0:r0 + rn, :].reshape(2, rn, 2, 4, 512) \
            .transpose(0, 3, 1, 2, 4).reshape(BPC, rn, C)

    osr = [np.asarray(res.results[i]["os"]).reshape(2, 2, 576, 4 * C)
           for i in range(NCORES)]
    ye = np.concatenate([unq(o, 0, 0, ns1) for o in osr], axis=0)
    yo = np.concatenate([unq(o, 1, 0, ns2) for o in osr], axis=0)
    aa = np.concatenate([unq(o, 0, H, ns1) for o in osr], axis=0)
    bb = np.concatenate([unq(o, 1, H, ns2) for o in osr], axis=0)

    x_dct_trunc = np.empty((B, L, C), dtype=np.float32)
    x_dct_trunc[:, 0::2, :] = ye.astype(np.float32)
    x_dct_trunc[:, 1::2, :] = yo.astype(np.float32)
    a32 = aa.astype(np.float32)
    b32 = bb.astype(np.float32)
    state = np.empty((B, L, C), dtype=np.float16)
    state[:, :ns2, :] = (a32[:, :ns2] + b32).astype(np.float16)
    if ns1 > ns2:
        state[:, ns2:ns1, :] = aa[:, ns2:ns1, :]   # lone middle row, L odd
    state[:, ns1:, :] = (a32[:, :ns2] - b32).astype(np.float16)[:, ::-1, :]
    return state, x_dct_trunc


# revision 45
# speedup vs baseline: 1.0542x; 1.0019x over previous
"""Trainium2 Bass kernel for DCTLAVISBlip dc_transform (DCT -> truncate -> IDCT).

Math (symmetry-folded, from v2): DCT parity M[k, T-1-t] = (-1)^k M[k,t]
folds the input on the host (u = xf+xr, v = xf-xr), halving the MACs.
Device runs Wu = [Me; pad; Pe'] and Wv = [Mo; pad; Po'] ([575, 288])
against u/v; y rows and raw a/b state halves ship as f16; the host does
the row interleave and the a+-b combine.  ~119-120us vs the 130us v2
baseline; PE busy ~95us of it (pass-count is within ~10% of the
M-row x K-tile lower bound for this shape, and deeper DCT factorization
levels fragment on the 128-lane granularity -- measured matmul cost is
flat ~240ns/512-col pass for any K<=128, so only pass count matters).

DMA/schedule structure (what the iterations v3-v9 established):
  1. v ships as fp8 e3m4 (4 mantissa bits), u as f16.  The PE accepts
     mixed f16-weight x fp8-moving matmuls; error goes 7e-4 -> 1.3e-2
     (tolerance 2e-2).  Both-sides e3m4 measured 2.1e-2 -- just over.
     Halves the v input bytes; v runs FIRST so the cheap kicks open the
     pipeline.
  2. ONE output DRAM tensor os[2, 2, 576, 4C] (wave-row x 4-batch
     layout, stage cols (ni, bi, 512)): ONE dma_start per (q, t,
     m-tile) = 20 calls of ~1MB with 8KB-contiguous DRAM lines.
     DIRECT2D issue costs 0.6-3us per call on a sequencer, so fewer,
     fatter calls beat many small ones; descriptors of one call fan
     out across all 16 SDMA engines.
  3. Output issue alternates the sync and gpsimd rings; every third
     call goes via the scalar (Act) ring DELAYED one wave, so its
     issue never blocks the scalar drain copies (that coupling cost
     v3 ~16us of PSUM stalls).  The last 4 waves ship each n-half as
     soon as its drains finish, across both free rings.
  4. Inputs stream on the scalar ring (weights on sync, v-first);
     the first wave's two kicks ship whole tiles (4KB-contiguous
     descriptor lines -- 1KB column-kicks measured descriptor-bound).
  5. PE warmup (memset + 18 matmuls) covers the HAM clock-gate window
     (~3.4us) during the input DMA head; first m-tile ramps in
     2-batch PSUM groups.  K=288 = 2x128 + 32-row remainder, the
     remainders of 4 batches packed on one 128-partition tile and
     co-executed on PE row-quarters via tile_position (the 4-way
     group costs ~1 pass instead of 4).
"""

import numpy as np

B, T, C = 64, 576, 1024
H = T // 2                   # 288, folded K
NCORES = 8
BPC = B // NCORES            # batches per core
Q = 0.8

_CACHED = {}


def _dct_mat(N):
    n = np.arange(N)
    Mm = np.cos(np.pi * (2 * n[None, :] + 1) * n[:, None] / (2 * N))
    s = np.full(N, np.sqrt(2.0 / N))
    s[0] = np.sqrt(1.0 / N)
    return s[:, None] * Mm          # float64


def _build_weights(L):
    """Wu [H+ns1, 288] = [Me; pad; Pe'], Wv [H+ns2, 288] = [Mo; pad; Po'].
    The y block is zero-padded up to H=288 rows so the state block starts at
    a 32-aligned PSUM partition in every m-tile."""
    M64 = _dct_mat(T)
    Mi = _dct_mat(L)
    ke = np.arange(0, L, 2)
    ko = np.arange(1, L, 2)
    Pe = np.einsum('kj,kt->jt', Mi[ke, :], M64[ke, :])
    Po = np.einsum('kj,kt->jt', Mi[ko, :], M64[ko, :])
    ns1 = (L + 1) // 2
    ns2 = L // 2
    pe_u = np.zeros((H - len(ke), H))
    pe_v = np.zeros((H - len(ko), H))
    Wu = np.concatenate([M64[ke][:, :H], pe_u, Pe[:ns1, :H]], axis=0)
    Wv = np.concatenate([M64[ko][:, :H], pe_v, Po[:ns2, :H]], axis=0)
    return Wu, Wv


def _build_nc(L):
    """Bass program for truncation length L (574 for the seed-0 input).

    Inputs host-packed as in v2:
      xu/xv  [2, 2, 128, 4C] f16: (q, ki, p, (b c))
      xur/xvr [2, 128, C]: K-remainder rows of 4 batches packed on partitions
      wub/wvb [128, 2M]: cols (ki m); wur/wvr [128, M]: rem rows 4x-replic.
    Outputs (v3): yy/ss [2, L, 4, C] f16 -- quad-major so one dma_start per
    (q, t, m-tile, dest) ships 4 batches with 8KB-contiguous DRAM lines.
    """
    import concourse.bacc as bacc
    import concourse.mybir as mybir
    import concourse.tile as tile

    f16 = mybir.dt.float16
    f32 = mybir.dt.float32

    ns1 = (L + 1) // 2
    ns2 = L // 2
    MU = H + ns1
    MV = H + ns2
    MW = {"u": MU, "v": MV}
    YB = {"u": ns1, "v": ns2}         # y rows per transform
    NT = [(0, 512), (512, 512)]
    MM = max(MU, MV)
    MT = [(m0, min(128, MM - m0)) for m0 in range(0, MM, 128)]

    f8 = mybir.dt.float8e3

    nc = bacc.Bacc("TRN2", target_bir_lowering=False, debug=False,
                   num_devices=NCORES)
    xu = nc.dram_tensor("xu", [2, 2, 128, 4 * C], f16, kind="ExternalInput")
    xv = nc.dram_tensor("xv", [2, 2, 128, 4 * C], f8, kind="ExternalInput")
    xur = nc.dram_tensor("xur", [2, 128, C], f16, kind="ExternalInput")
    xvr = nc.dram_tensor("xvr", [2, 128, C], f8, kind="ExternalInput")
    wub = nc.dram_tensor("wub", [128, 2 * MU], f16, kind="ExternalInput")
    wvb = nc.dram_tensor("wvb", [128, 2 * MV], f16, kind="ExternalInput")
    wur = nc.dram_tensor("wur", [128, MU], f16, kind="ExternalInput")
    wvr = nc.dram_tensor("wvr", [128, MV], f16, kind="ExternalInput")
    # one output tensor: plane t=0 holds [y-u rows; pad; a rows], t=1 holds
    # [y-v rows; pad; b rows] -- one dma_start per (q, t, m-tile)
    os_ = nc.dram_tensor("os", [2, 2, 576, 4 * C], f16,
                         kind="ExternalOutput")
    XD = {"u": (xu, xur, wub, wur), "v": (xv, xvr, wvb, wvr)}

    with tile.TileContext(nc) as tc:
        with (
            tc.tile_pool(name="wpool", bufs=1) as wpool,
            tc.tile_pool(name="xpool", bufs=1) as xpool,
            tc.tile_pool(name="opool", bufs=8) as opool,
            tc.tile_pool(name="ps", bufs=8, space="PSUM") as ps,
        ):
            # --- warmup immediately: memset on vector (idle at start) ---
            wz = wpool.tile([128, 128], f16, tag="wz", name="wz")
            nc.vector.memset(wz[:], 0.0)
            pwarm = ps.tile([128, 512], f32, tag="pt", name="pt")
            for _ in range(14):
                nc.tensor.matmul(pwarm[:, 0:128], wz[:], wz[:],
                                 start=True, stop=True)

            # --- input kicks, first-use order, spread across engines ---
            xt, rt, wt, wr = {}, {}, {}, {}

            def load_w(t, eng):
                _, _, wd, wrd = XD[t]
                w_ = wpool.tile([128, 2 * MW[t]], f16, tag=f"w{t}",
                                name=f"w{t}")
                eng.dma_start(w_[:], wd[:, :])
                wt[t] = w_
                w_ = wpool.tile([128, MW[t]], f16, tag=f"w{t}r",
                                name=f"w{t}r")
                eng.dma_start(w_[:], wrd[:, :])
                wr[t] = w_

            # v weights: the first m-tile's ki0 slice (32KB) ships alone so
            # the first matmul's weight dependency clears immediately; the
            # rest and the rem weights follow after the first kicks
            _, _, wvd, wvrd = XD["v"]
            wv_ = wpool.tile([128, 2 * MV], f16, tag="wv", name="wv")
            # BOTH of m-tile 0's weight slices (ki0 cols [0:128], ki1 cols
            # [MV:MV+128]) ship first, so wave 0 never waits on weights
            nc.sync.dma_start(wv_[:, 0:128], wvd[:, 0:128])
            nc.sync.dma_start(wv_[:, MV:MV + 128], wvd[:, MV:MV + 128])
            wt["v"] = wv_
            # first wave's (q0, v) kicks: 2KB-contiguous-line halves in
            # first-use order (the ramp needs [0:2C] of both ki)
            xdt = {"u": f16, "v": f8}
            xd, rd, _, _ = XD["v"]
            for ki, eng in ((0, nc.scalar), (1, nc.sync)):
                x_ = xpool.tile([128, 4 * C], f8, tag=f"xv0{ki}",
                                name=f"xv0{ki}")
                xt[("v", 0, ki)] = x_
                eng.dma_start(x_[:, 0:2 * C], xd[0, ki, :, 0:2 * C])
            r_ = xpool.tile([128, C], f8, tag="xvr0", name="xvr0")
            nc.gpsimd.dma_start(r_[:], rd[0, :, :])
            rt[("v", 0)] = r_
            for ki in range(2):
                nc.scalar.dma_start(xt[("v", 0, ki)][:, 2 * C:4 * C],
                                    xd[0, ki, :, 2 * C:4 * C])
            # weight bulk after the first-wave inputs
            nc.sync.dma_start(wv_[:, 128:MV], wvd[:, 128:MV])
            nc.sync.dma_start(wv_[:, MV + 128:2 * MV],
                              wvd[:, MV + 128:2 * MV])
            wvr_ = wpool.tile([128, MV], f16, tag="wvr", name="wvr")
            nc.gpsimd.dma_start(wvr_[:], wvrd[:, :])
            wr["v"] = wvr_
            load_w("u", nc.sync)
            for t in ("v", "u"):
                for q in range(2):
                    if q == 0 and t == "v":
                        continue
                    xd, rd, _, _ = XD[t]
                    for ki in range(2):
                        x_ = xpool.tile([128, 4 * C], xdt[t],
                                        tag=f"x{t}{q}{ki}",
                                        name=f"x{t}{q}{ki}")
                        nc.scalar.dma_start(x_[:, 0:2 * C],
                                            xd[q, ki, :, 0:2 * C])
                        xt[(t, q, ki)] = x_
                    r_ = xpool.tile([128, C], xdt[t], tag=f"x{t}r{q}",
                                    name=f"x{t}r{q}")
                    nc.gpsimd.dma_start(r_[:], rd[q, :, :])
                    rt[(t, q)] = r_
                    for ki in range(2):
                        nc.scalar.dma_start(xt[(t, q, ki)][:, 2 * C:4 * C],
                                            xd[q, ki, :, 2 * C:4 * C])

            def vcopy(dst, src):
                nc.vector.tensor_copy(dst, src)

            def scopy(dst, src):
                nc.scalar.copy(dst, src)

            oengs = [nc.sync, nc.gpsimd]
            ok_i = 0     # output call counter (engine rotation)
            pending = None   # delayed output call issued via scalar ring

            # --- compute waves: (t, q, m), 2 n-halves x 4 batches ---
            # BOTH v quads first: the first 10 waves need only 2.35MB of
            # fp8 input, while all of u (f16) streams during v's compute
            NW = 20          # total waves
            for t in ("v", "u"):
                for q in range(2):
                    ti = 0 if t == "u" else 1
                    mw = MW[t]
                    for mi, (m0, mm) in enumerate(MT):
                        mmt = min(mm, mw - m0)
                        if mmt <= 0:
                            continue
                        stage = opool.tile([128, 4 * C], f16,
                                           tag="o", name="o")
                        for ni, (n0, nn) in enumerate(NT):
                            # ramp: first m-tile of the run goes in 2-bank
                            # halves so compute starts on half the inputs
                            groups = ([(0, 1), (2, 3)]
                                      if (q == 0 and t == "v" and mi == 0)
                                      else [(0, 1, 2, 3)])
                            pts = {}
                            for grp in groups:
                                for bi in grp:
                                    pts[bi] = ps.tile([128, 512], f32,
                                                      tag="pt", name="pt")
                                for ki in range(2):
                                    wsl = wt[t][:, ki * mw + m0:
                                                ki * mw + m0 + mmt]
                                    for bi in grp:
                                        nc.tensor.matmul(
                                            pts[bi][0:mmt, :],
                                            wsl,
                                            xt[(t, q, ki)][:, bi * C + n0:
                                                           bi * C + n0 + nn],
                                            start=(ki == 0), stop=False)
                                for bi in grp:
                                    nc.tensor.matmul(
                                        pts[bi][0:mmt, :],
                                        wr[t][32 * bi:32 * bi + 32,
                                              m0:m0 + mmt],
                                        rt[(t, q)][32 * bi:32 * bi + 32,
                                                   n0:n0 + nn],
                                        start=False, stop=True,
                                        tile_position=(32 * bi, 0))
                            # stage columns laid out (ni, bi, 512): each
                            # n-half is contiguous, so tail waves can ship
                            # a half as soon as its drains complete
                            for bi in range(4):
                                cp = vcopy if bi % 2 == 0 else scopy
                                c0 = ni * 2 * C + bi * 512
                                cp(stage[0:mmt, c0:c0 + nn],
                                   pts[bi][0:mmt, :])
                                if ok_i >= NW - 4 and bi % 2 == 1:
                                    # tail waves: ship each 1KB batch-pair
                                    # chunk as soon as its 2 drains land
                                    cc = ni * 2 * C + (bi - 1) * 512
                                    d = os_[q, ti, m0:m0 + mmt,
                                            cc:cc + 2 * 512]
                                    oengs[(2 * ni + bi // 2) % 2].dma_start(
                                        d, stage[0:mmt, cc:cc + 2 * 512])
                        # ONE dma_start per wave ships y+state rows of all
                        # 4 batches.  Rotation sync/gpsimd immediate; every
                        # third call goes via the scalar (Act) ring delayed
                        # ONE wave so its issue never blocks scalar drains.
                        if pending is not None:
                            nc.scalar.dma_start(*pending)
                            pending = None
                        d = os_[q, ti, m0:m0 + mmt, :]
                        if ok_i >= NW - 4:
                            pass     # shipped per n-half above
                        elif ok_i % 3 == 2:
                            pending = (d, stage[0:mmt, :])
                        else:
                            oengs[ok_i % 3].dma_start(d, stage[0:mmt, :])
                        ok_i += 1
            if pending is not None:
                nc.scalar.dma_start(*pending)
                pending = None

    nc.finalize()
    return nc


def _get_nc(L):
    key = ("nc3", L)
    if key not in _CACHED:
        _CACHED[key] = _build_nc(L)
    return _CACHED[key]


def _ensure_trace_hook_safe():
    """If BASS_TRACE is set in the environment, run_bass_kernel_spmd imports
    antenv.axon_hooks, which may not exist. Install a working ctypes-based
    shim when possible, else disable tracing so the run cannot crash."""
    import os
    import sys
    import types

    if not os.environ.get("BASS_TRACE"):
        return
    try:
        import antenv.axon_hooks  # noqa: F401
        return
    except ImportError:
        pass
    try:
        from trn_agent_boot.trn_boot import _ntff_profile_via_ctypes
        hooks = types.ModuleType("antenv.axon_hooks")
        hook = _ntff_profile_via_ctypes("/opt/axon/libaxon_pjrt.so")
        hooks.get_axon_ntff_profile_hook = lambda: hook
        hooks.set_axon_ntff_profile_hook = lambda h: None
        sys.modules["antenv.axon_hooks"] = hooks
    except Exception:
        os.environ["BASS_NEVER_TRACE"] = "1"


def kernel(x: np.ndarray):
    from concourse.bass_utils import run_bass_kernel_spmd

    _ensure_trace_hook_safe()
    x = np.ascontiguousarray(np.asarray(x, dtype=np.float32))
    assert x.shape == (B, T, C)

    # ---- host: data-dependent truncation length L (tiny, exact math) ----
    M64 = _dct_mat(T)
    xbar = x.astype(np.float64).mean(axis=(0, 2))
    vq = np.abs(M64 @ xbar)
    thr = np.abs(np.quantile(vq, Q))
    idxs = np.where(vq > thr)[0]
    last_index = int(idxs[-1]) if idxs.size > 0 else -1
    L = last_index if last_index >= 0 else T - 1

    ns1 = (L + 1) // 2
    Wu, Wv = _build_weights(L)              # [H+ns1, 288], [H+ns2, 288]
    wu16 = np.ascontiguousarray(Wu.T).astype(np.float16)   # [288, H+ns1]
    wv16 = np.ascontiguousarray(Wv.T).astype(np.float16)

    # ---- host: fold input (u ships f16, v ships fp8 e3m4) ----
    import ml_dtypes
    xf = x[:, :H, :]
    xr = x[:, T - 1:H - 1:-1, :]
    u16 = (xf + xr).astype(np.float16)
    v16 = (xf - xr).astype(ml_dtypes.float8_e3m4)

    nc = _get_nc(L)

    def pack_x(z16):
        # [BPC,288,C] -> [2,2,128,4C] (q, ki, p, (b c)) + rem [2,128,C]
        full = z16[:, :256].reshape(2, 4, 2, 128, C)
        full = np.ascontiguousarray(full.transpose(0, 2, 3, 1, 4)
                                    ).reshape(2, 2, 128, 4 * C)
        remn = np.ascontiguousarray(z16[:, 256:288]).reshape(2, 128, C)
        return full, remn

    def pack_w(w16):
        # [288, M] -> [128, 2M] cols (ki m) + rem rows replicated [128, M]
        full = np.ascontiguousarray(w16[:256].reshape(2, 128, w16.shape[1])
                                    .transpose(1, 0, 2)
                                    ).reshape(128, 2 * w16.shape[1])
        remn = np.ascontiguousarray(np.tile(w16[256:288], (4, 1)))
        return full, remn

    wub_h, wur_h = pack_w(wu16)
    wvb_h, wvr_h = pack_w(wv16)
    in_maps = []
    for i in range(NCORES):
        xu_h, xur_h = pack_x(u16[i * BPC:(i + 1) * BPC])
        xv_h, xvr_h = pack_x(v16[i * BPC:(i + 1) * BPC])
        in_maps.append({"xu": xu_h, "xv": xv_h, "xur": xur_h, "xvr": xvr_h,
                        "wub": wub_h, "wvb": wvb_h,
                        "wur": wur_h, "wvr": wvr_h})
    res = run_bass_kernel_spmd(nc, in_maps, list(range(NCORES)))
    _CACHED["last_exec_time_ns"] = res.exec_time_ns

    # device layout os [2(q), 2(t), 576, 4, C]:
    #   t=0 rows [0:ns1]=y-even, [H:H+ns1]=a;  t=1 [0:ns2]=y-odd, [H:H+ns2]=b
    ns2 = L // 2

    def unq(o, tp, r0, rn):
        # stage cols (ni, bi, 512): [2, rn, 2, 4, 512] -> [BPC, rn, C]
        return o[:, tp, r0:r0 + rn, :].reshape(2, rn, 2, 4, 512) \
            .transpose(0, 3, 1, 2, 4).reshape(BPC, rn, C)

    osr = [np.asarray(res.results[i]["os"]).reshape(2, 2, 576, 4 * C)
           for i in range(NCORES)]
    ye = np.concatenate([unq(o, 0, 0, ns1) for o in osr], axis=0)
    yo = np.concatenate([unq(o, 1, 0, ns2) for o in osr], axis=0)
    aa = np.concatenate([unq(o, 0, H, ns1) for o in osr], axis=0)
    bb = np.concatenate([unq(o, 1, H, ns2) for o in osr], axis=0)

    x_dct_trunc = np.empty((B, L, C), dtype=np.float32)
    x_dct_trunc[:, 0::2, :] = ye.astype(np.float32)
    x_dct_trunc[:, 1::2, :] = yo.astype(np.float32)
    a32 = aa.astype(np.float32)
    b32 = bb.astype(np.float32)
    state = np.empty((B, L, C), dtype=np.float16)
    state[:, :ns2, :] = (a32[:, :ns2] + b32).astype(np.float16)
    if ns1 > ns2:
        state[:, ns2:ns1, :] = aa[:, ns2:ns1, :]   # lone middle row, L odd
    state[:, ns1:, :] = (a32[:, :ns2] - b32).astype(np.float16)[:, ::-1, :]
    return state, x_dct_trunc
